# revision 26
# baseline (speedup 1.0000x reference)
"""Trainium2 Bass kernel for nn_ArrivalTime (sparse attention over 24 timeslots).

Math refactoring (exact, up to fp reassociation):
  query = [user_pref[user], timeslot[hour]] has only 64 distinct user rows and
  24 distinct time rows, so
    scores[n,h,t] = US[b(n), h, t] + TS[hour[n], h, t]
  with tiny host-precomputed tables; US is folded into a per-batch-row A-table
  (the stream carries a constant ones-row), so no activation bias is needed.
  Masking adds -1e9 where hour_mask==1.  Softmax per head over t (24).
  Output: out[n,:] = attn[n,:] @ vproj + bu, vproj[(h,t),d] = v[h,t,:]@Wu[d,h*HD:]^T.

Device pipeline (per core, transposed layout: tokens on the free dim), one
iteration per batch row (512 tokens), matmul operands bf16:
  PE : ps_s = table_b^T @ stream    (one-hot hour + mask + ones rows, K=49)
  ACT: p = exp(ps_s)                (row 96 = exp(0) = 1 -> carries bu)
  PE : ps_z = seg2^T @ p            (per-head sums replicated, written into a
                                     [96, 2S] wide psum so ln/exp batch x2)
  ACT: lnz = ln(zw); r = exp(-lnz)  (ONE ln + ONE exp(-x) per TWO rows --
                                     ACT cost is free-dim bound)
  DVE: p[:96] *= r slice            (bf16 all-SBUF -> 2x mode)
  PE : ps_o = vproj_ext^T @ p       (two halves into one 2-bank f32 psum;
                                     vproj row 96 = bu)
  DVE: ot = ps_o                    (single [128,2S] f32->bf16 cast)
  SP : one bf16 output DMA per iteration.
All DMAs are issued from the SP ring (16 DMA engines; the ACT hwdge ring maps
to a single engine and is ~5x slower for multi-descriptor transfers).  GpSimd
is never used: it cannot touch PSUM and its SBUF traffic slows every other
engine ~20%.  m2 lags 3 iterations; standalone wait_ge synchronization.

Sharding: data-parallel over batch, 8 batch rows (= 8 x 512 tokens) per core.
"""

import os
import numpy as np

B, S, D, H, HD, T = 64, 512, 256, 4, 64, 24
NCORES = 8
BPC = B // NCORES  # batch rows per core
HT = H * T  # 96
K1 = 2 * T + 1  # 49 stream rows: one-hot hour + mask + ones
MASK_NEG = -1.0e9
TW = HT + 1  # 97: table columns / p partitions

# vpseg bf16 constant tensor [97, VW]: vproj_ext then seg2
C_VP = 0
C_SEG2 = C_VP + D
VW = C_SEG2 + HT


def _host_tables(timeslot_embedded, user, hour, hour_mask, user_pref,
                 Wq, bq, Wk, bk, Wv, bv, Wu, bu):
    import ml_dtypes
    f32 = np.float32
    bf16 = ml_dtypes.bfloat16
    ts_e = np.asarray(timeslot_embedded, f32)          # [T, D]
    user = np.asarray(user).astype(np.int64)           # [B]
    hour = np.asarray(hour).astype(np.int64)           # [B, S]
    hour_mask = np.asarray(hour_mask)                  # [B, S, T]
    Wq = np.asarray(Wq, f32); bq = np.asarray(bq, f32)
    Wk = np.asarray(Wk, f32); bk = np.asarray(bk, f32)
    Wv = np.asarray(Wv, f32); bv = np.asarray(bv, f32)
    Wu = np.asarray(Wu, f32); bu = np.asarray(bu, f32)

    Wq_u, Wq_t = Wq[:, :, :D], Wq[:, :, D:]
    k_ = np.einsum('td,hkd->htk', ts_e, Wk) + bk[:, None, :]   # [H,T,HD]
    v_ = np.einsum('td,hkd->htk', ts_e, Wv) + bv[:, None, :]
    time_q = np.einsum('td,hkd->thk', ts_e, Wq_t)              # [T,H,HD]
    upref = np.asarray(user_pref, f32)[user]                   # [B,D]
    user_q = np.einsum('bd,hkd->bhk', upref, Wq_u) + bq[None]  # [B,H,HD]
    scale = f32(1.0 / np.sqrt(HD))
    TS = (np.einsum('thk,hsk->ths', time_q, k_) * scale).reshape(T, HT)
    US = (np.einsum('bhk,hsk->bhs', user_q, k_) * scale).reshape(B, HT)
    vproj = np.einsum('htk,dhk->htd', v_, Wu.reshape(D, H, HD)).reshape(HT, D)

    # per-batch-row tables [K1, TW]: rows 0..23 TS, rows 24..47 mask additive,
    # row 48 = US_b (ones-row of the stream); col 96 = 0 everywhere
    maskrows = np.tile(np.eye(T, dtype=f32), (1, H)) * f32(MASK_NEG)
    tabs_cores = []
    for c in range(NCORES):
        tc = np.zeros((K1, BPC * TW), f32)
        for j in range(BPC):
            b = c * BPC + j
            tc[:T, j * TW:j * TW + HT] = TS
            tc[T:2 * T, j * TW:j * TW + HT] = maskrows
            tc[2 * T, j * TW:j * TW + HT] = US[b]
        tabs_cores.append(tc.astype(bf16))

    seg2 = np.kron(np.eye(H, dtype=f32), np.ones((T, T), f32))  # [HT, HT]
    vs = np.zeros((TW, VW), f32)
    vs[:HT, C_VP:C_VP + D] = vproj
    vs[HT, C_VP:C_VP + D] = bu
    vs[:HT, C_SEG2:C_SEG2 + HT] = seg2
    vpseg_bf = vs.astype(bf16)

    # per-core streams [BPC, K1, S] bf16: one-hot(hour) + mask^T + ones rows
    eyeT = np.eye(T, dtype=f32)
    streams = []
    for c in range(NCORES):
        hb = hour[c * BPC:(c + 1) * BPC]                       # [BPC, S]
        mb = hour_mask[c * BPC:(c + 1) * BPC]                  # [BPC, S, T]
        st = np.empty((BPC, K1, S), f32)
        st[:, :T, :] = eyeT[hb].transpose(0, 2, 1)
        st[:, T:2 * T, :] = mb.astype(f32).transpose(0, 2, 1)
        st[:, 2 * T, :] = 1.0
        streams.append(st.astype(bf16))
    return tabs_cores, vpseg_bf, streams


def _build_program():
    import concourse.bass as bass
    import concourse.mybir as mybir
    from contextlib import ExitStack

    class _NoBarrierBlock(bass.BassBlock):
        # The stock Block.__exit__ emits per-engine drains plus a full
        # all-engine semaphore barrier whose wakeup costs ~6-8us of tail.
        # Output completion is already guaranteed by the explicit ot_sem
        # waits on the sync engine.
        def __exit__(self, exc_type, exc_val, exc_tb):
            if exc_type is None:
                for engine, last_body in self.last_body.items():
                    with self.bass.body(last_body, parent=self.bass.cur_bb,
                                        allow_existing_parent=True):
                        engine.br(self.end_bb)
                self.bass.switch_bb(self.end_bb)

    f32 = mybir.dt.float32
    bf16 = mybir.dt.bfloat16
    nc = bass.Bass("TRN2")
    stream_d = nc.declare_dram_parameter("stream", [BPC, K1, S], bf16,
                                         isOutput=False)
    tabs_d = nc.declare_dram_parameter("tabs", [K1, BPC * TW], bf16,
                                       isOutput=False)
    vpseg_d = nc.declare_dram_parameter("vpseg", [TW, VW], bf16,
                                        isOutput=False)
    out_d = nc.declare_dram_parameter("out", [BPC, D, S], bf16, isOutput=True)

    Exp = mybir.ActivationFunctionType.Exp
    Ln = mybir.ActivationFunctionType.Ln
    LAG = 4  # m2 lags 4 rows so m2 overlaps the next pair's ln/expneg

    with ExitStack() as ctx:
        ec = ctx.enter_context
        tabs_sb = ec(nc.sbuf_tensor("tabs_sb", [K1, BPC * TW], bf16))
        vpseg_sb = ec(nc.sbuf_tensor("vpseg_sb", [TW, VW], bf16))
        sts = [ec(nc.sbuf_tensor(f"st{j}", [K1, S], bf16)) for j in range(BPC)]
        ps = [ec(nc.sbuf_tensor(f"p{j}", [TW, S], bf16)) for j in range(6)]
        lnz_sb = ec(nc.sbuf_tensor("lnz_sb", [HT, 2 * S], f32))
        r_sb = ec(nc.sbuf_tensor("r_sb", [HT, 2 * S], bf16))
        ots = [ec(nc.sbuf_tensor(f"ot{j}", [128, 2 * S], bf16))
               for j in range(3)]
        warm_sb = ec(nc.sbuf_tensor("warm_sb", [128, S], bf16))
        ps_ss = [ec(nc.psum_tensor(f"ps_s{j}", [TW, S], f32))
                 for j in range(2)]
        zw = ec(nc.psum_tensor("zw", [HT, 2 * S], f32))
        ps_os = [ec(nc.psum_tensor(f"ps_o{j}", [128, 2 * S], f32))
                 for j in range(2)]
        warm_sem = ec(nc.semaphore("warm_sem"))
        c_sem = ec(nc.semaphore("c_sem"))     # tabs DMA
        v_sem = ec(nc.semaphore("v_sem"))     # vpseg DMA
        st_sems = [ec(nc.semaphore(f"st_sem{j}")) for j in range(BPC)]
        pe_sem = ec(nc.semaphore("pe_sem"))
        act_sem = ec(nc.semaphore("act_sem"))
        dve_sem = ec(nc.semaphore("dve_sem"))
        ot_sems = [ec(nc.semaphore(f"ot_sem{j}")) for j in range(BPC)]
        nc.check_frozen()
        block = ec(_NoBarrierBlock(nc, f"block_{nc.next_id()}"))
        nc.cur_block = block

        vproj = vpseg_sb[:, C_VP:C_VP + D]
        seg2 = vpseg_sb[:, C_SEG2:C_SEG2 + HT]

        def tab(i):
            return tabs_sb[:, i * TW:(i + 1) * TW]

        pe_tick = {}
        act_tick = {}
        dve_tick = {}
        _cnt = {'pe': 0, 'act': 0, 'dve': 0}

        def _rec(tickmap, cnt_key, key):
            _cnt[cnt_key] += 1
            tickmap[key] = _cnt[cnt_key]

        # ACT emission order: exp runs one row ahead; the pair ln/expneg is
        # emitted after exp_{2m+2} so it never blocks the next exp.
        # pairs 0..BPC//2-2 batched; the last two rows are handled per-row
        # (lnr/expnegr on a [96,S] half) to shorten the end-of-kernel chain
        act_order = []
        for i in range(BPC):
            act_order.append(('exp', i))
            if i >= 2 and i % 2 == 0 and i // 2 - 1 < BPC // 2 - 1:
                act_order.append(('ln', i // 2 - 1))
            if i >= 3 and i % 2 == 1 and i // 2 - 1 < BPC // 2 - 1:
                act_order.append(('expneg', i // 2 - 1))
        act_order.append(('lnr', BPC - 2))
        act_order.append(('expnegr', BPC - 2))
        act_order.append(('lnr', BPC - 1))
        act_order.append(('expnegr', BPC - 1))
        for key in act_order:
            _rec(act_tick, 'act', key)

        # DVE emission order: ocopy first (its data arrives earlier), then mul
        dve_order = []
        for i in range(BPC + LAG):
            if i >= LAG:
                k = i - LAG
                if k == BPC - 1:
                    dve_order.append(('ocopyA', k))
                    dve_order.append(('ocopyB', k))
                else:
                    dve_order.append(('ocopy', k))
            if i < BPC:
                dve_order.append(('mul', i))
        for key in dve_order:
            _rec(dve_tick, 'dve', key)

        @block.tensor
        def _(tensor):
            def mm(key, out, lhsT, rhs):
                tensor.matmul(out, lhsT, rhs,
                              start=True, stop=True).then_inc(pe_sem, 1)
                _rec(pe_tick, 'pe', key)

            # HAM warm-up: 5 x 512-col matmuls on memset scratch during the
            # input-DMA flight window release the PE clock throttle
            # (K=4/8 -> 8/8) right as the real work begins.  Results land in
            # ps_o[0] which m2_0 later overwrites with start=True.
            def warm_mm(n=S):
                tensor.matmul(ps_os[0][:, 0:n], warm_sb[:, 0:128],
                              warm_sb[:, 0:n], start=True, stop=True)

            tensor.wait_ge(warm_sem, 1)
            for _ in range(6):
                warm_mm()
            tensor.wait_ge(c_sem, 16)
            tensor.wait_ge(st_sems[0], 16)
            mm(('A', 0), ps_ss[0][:], tab(0), sts[0][:])
            for j in range(BPC + LAG):
                if j + 1 < BPC:                 # A_{j+1}
                    i = j + 1
                    tensor.wait_ge(st_sems[i], 16)
                    if j >= 1:
                        tensor.wait_ge(act_sem, act_tick[('exp', j - 1)])
                    mm(('A', i), ps_ss[i % 2][:], tab(i), sts[i][:])
                if j < LAG - 1:
                    # fill-phase HAM keep-alive: the first blocks only have
                    # 2 real matmuls and stall on the exp chain; idle >1
                    # MID window would re-throttle the PE clock
                    warm_mm(256)
                    warm_mm(256)
                if j == LAG:
                    # the first m2 group waits on the first ln/expneg chain
                    # (~1.5us); keep the PE clock released through it
                    warm_mm()
                    warm_mm()
                if 0 <= j - LAG < BPC:          # m2_{j-LAG}
                    i = j - LAG
                    tensor.wait_ge(dve_sem, dve_tick[('mul', i)])
                    if i == 0:
                        tensor.wait_ge(v_sem, 32)
                    if i >= 2:                  # ocopy_{i-2} freed ps_o
                        kk = i - 2
                        okey = ('ocopyB', kk) if kk == BPC - 1 else \
                            ('ocopy', kk)
                        tensor.wait_ge(dve_sem, dve_tick[okey])
                    mm(('m2a', i), ps_os[i % 2][:, 0:S],
                       vproj[:, 0:128], ps[i % 6][:])
                    mm(('m2b', i), ps_os[i % 2][:, S:2 * S],
                       vproj[:, 128:256], ps[i % 6][:])
                    if j > BPC:
                        # tail keep-alive: the next m2 pair waits on the
                        # per-row ln chain; ps_s has no readers by now
                        tensor.matmul(ps_ss[0][:, 0:S], warm_sb[:, 0:97],
                                      warm_sb[:], start=True, stop=True)
                if j < BPC:                     # hs_j
                    tensor.wait_ge(act_sem, act_tick[('exp', j)])
                    if j == 0:
                        tensor.wait_ge(v_sem, 32)
                    m = j // 2
                    if j >= 2:                  # previous reader freed zw half
                        if j >= BPC - 2:
                            tensor.wait_ge(act_sem,
                                           act_tick[('ln', BPC // 2 - 2)])
                        else:
                            tensor.wait_ge(act_sem, act_tick[('ln', m - 1)])
                    mm(('hs', j), zw[:, (j % 2) * S:(j % 2 + 1) * S],
                       seg2, ps[j % 6][:])

        @block.scalar
        def _(scalar):
            # preload the Exp/Ln PWP tables during the DMA flight window
            # (input is the preamble-initialized const-1.0 AP so CoreSim's
            # uninitialized-read check stays clean; scratch out into ot0)
            cap = nc.const_aps.aps[(f32, 1.0)]
            scalar.activation(lnz_sb[:4, 0:1], cap[0:4], Exp)
            scalar.activation(lnz_sb[:4, 1:2], cap[0:4], Ln)
            for key in act_order:
                kind, i = key
                if kind == 'exp':
                    scalar.wait_ge(pe_sem, pe_tick[('A', i)])
                    scalar.activation(ps[i % 6][:], ps_ss[i % 2][:],
                                      Exp).then_inc(act_sem, 1)
                elif kind == 'ln':
                    scalar.wait_ge(pe_sem, pe_tick[('hs', 2 * i + 1)])
                    if i >= 1:  # self-wait: expneg_{i-1}'s lnz reads drained
                        scalar.wait_ge(act_sem, act_tick[('expneg', i - 1)])
                    scalar.activation(lnz_sb[:], zw[:],
                                      Ln).then_inc(act_sem, 1)
                elif kind == 'expneg':
                    if i >= 1:                  # muls of pair i-1 freed r_sb
                        scalar.wait_ge(dve_sem, dve_tick[('mul', 2 * i - 1)])
                    # self-wait: ln_i's writes must have fully drained
                    scalar.wait_ge(act_sem, act_tick[('ln', i)])
                    scalar.activation(r_sb[:], lnz_sb[:], Exp,
                                      scale=-1.0).then_inc(act_sem, 1)
                elif kind == 'lnr':             # per-row ln, row i
                    sl = (i % 2) * S
                    scalar.wait_ge(pe_sem, pe_tick[('hs', i)])
                    if i == BPC - 2:  # last batched expneg's reads drained
                        scalar.wait_ge(act_sem,
                                       act_tick[('expneg', BPC // 2 - 2)])
                    scalar.activation(lnz_sb[:, sl:sl + S],
                                      zw[:, sl:sl + S],
                                      Ln).then_inc(act_sem, 1)
                else:  # expnegr, row i
                    sl = (i % 2) * S
                    if i == BPC - 2:            # muls of the last batched
                        scalar.wait_ge(dve_sem,  # pair freed r_sb
                                       dve_tick[('mul', BPC - 3)])
                    scalar.wait_ge(act_sem, act_tick[('lnr', i)])
                    scalar.activation(r_sb[:, sl:sl + S],
                                      lnz_sb[:, sl:sl + S], Exp,
                                      scale=-1.0).then_inc(act_sem, 1)

        @block.vector
        def _(vector):
            vector.memset(warm_sb[:], 0.0).then_inc(warm_sem, 1)
            for key in dve_order:
                kind, i = key
                if kind == 'mul':
                    if i >= BPC - 2:
                        vector.wait_ge(act_sem, act_tick[('expnegr', i)])
                    else:
                        vector.wait_ge(act_sem, act_tick[('expneg', i // 2)])
                    sl = (i % 2) * S
                    vector.tensor_mul(ps[i % 6][:HT, :], ps[i % 6][:HT, :],
                                      r_sb[:, sl:sl + S]).then_inc(dve_sem, 1)
                elif kind == 'ocopy':
                    vector.wait_ge(pe_sem, pe_tick[('m2b', i)])
                    if i >= 3:
                        vector.wait_ge(ot_sems[i - 3], 16)
                    vector.tensor_copy(ots[i % 3][:],
                                       ps_os[i % 2][:]).then_inc(dve_sem, 1)
                elif kind == 'ocopyA':
                    vector.wait_ge(pe_sem, pe_tick[('m2a', i)])
                    if i >= 3:
                        vector.wait_ge(ot_sems[i - 3], 16)
                    vector.tensor_copy(ots[i % 3][:, 0:S],
                                       ps_os[i % 2][:, 0:S]).then_inc(
                                           dve_sem, 1)
                else:  # ocopyB
                    vector.wait_ge(pe_sem, pe_tick[('m2b', i)])
                    vector.tensor_copy(ots[i % 3][:, S:2 * S],
                                       ps_os[i % 2][:, S:2 * S]).then_inc(
                                           dve_sem, 1)

        @block.sync
        def _(sync):
            sync.dma_start(sts[0][:], stream_d[0]).then_inc(st_sems[0], 16)
            sync.dma_start(tabs_sb[:], tabs_d[:]).then_inc(c_sem, 16)
            sync.dma_start(sts[1][:], stream_d[1]).then_inc(st_sems[1], 16)
            sync.dma_start(vpseg_sb[0:49, :],
                           vpseg_d[0:49, :]).then_inc(v_sem, 16)
            sync.dma_start(vpseg_sb[49:TW, :],
                           vpseg_d[49:TW, :]).then_inc(v_sem, 16)
            for i in range(2, BPC):
                sync.dma_start(sts[i][:], stream_d[i]).then_inc(st_sems[i], 16)
            for k in range(BPC - 1):
                sync.wait_ge(dve_sem, dve_tick[('ocopy', k)])
                dest = out_d[k, :, :].rearrange("(h p) s -> p h s", h=2)
                src = ots[k % 3][:, :].rearrange("p (h s) -> p h s", h=2)
                sync.dma_start(dest, src).then_inc(ot_sems[k], 16)
            k = BPC - 1
            sync.wait_ge(dve_sem, dve_tick[('ocopyA', k)])
            sync.dma_start(out_d[k, 0:128, :],
                           ots[k % 3][:, 0:S]).then_inc(ot_sems[k], 16)
            sync.wait_ge(dve_sem, dve_tick[('ocopyB', k)])
            sync.dma_start(out_d[k, 128:256, :],
                           ots[k % 3][:, S:2 * S]).then_inc(ot_sems[k], 16)
            for k in range(BPC - 1):
                sync.wait_ge(ot_sems[k], 16)
            sync.wait_ge(ot_sems[BPC - 1], 32)

    return nc


def _run(inputs, trace=False):
    import sys
    if "/opt/trn_rl_repo" not in sys.path:
        sys.path.insert(0, "/opt/trn_rl_repo")
    from concourse.bass_utils import run_bass_kernel_spmd

    tabs_cores, vpseg_bf, streams = _host_tables(**inputs)
    nc = _build_program()
    in_maps = [
        {"stream": streams[c], "tabs": tabs_cores[c], "vpseg": vpseg_bf}
        for c in range(NCORES)
    ]
    res = run_bass_kernel_spmd(nc, in_maps, core_ids=list(range(NCORES)),
                               trace=trace)
    out_full = np.empty((B, S, D), np.float32)
    for c in range(NCORES):
        oc = res.results[c]["out"]  # [BPC, D, S] bf16
        out_full[c * BPC:(c + 1) * BPC] = \
            oc.astype(np.float32).transpose(0, 2, 1)
    return out_full, res


def kernel(**inputs):
    trace = bool(int(os.environ.get("BASS_KERNEL_TRACE", "0")))
    out, _ = _run(inputs, trace=trace)
    return out


def kernel_profiled(**inputs):
    out, res = _run(inputs, trace=True)
    return out, res
